# revision 22
# baseline (speedup 1.0000x reference)
"""Trainium2 Bass kernel for nn_BCHConv2D (complex harmonic conv + bispectrum).

Strategy (8 NeuronCores, data-parallel over batch B=8):
  host: build complex-harmonic filters from w+atoms -> fdA [128, 7*208],
        fdB [96, 7*208] (kh-reversed contiguous blocks), transpose each
        batch image to (H, C, W), replicate bias.
  core: 7x7x32 -> 208ch conv; PSUM banks hold PAIRS of output rows
        [122, 2*208]; per input row the two kh-adjacent filter blocks are
        streamed as one 416-wide moving operand (halves matmul count).
        PSUM -> fp16 SBUF (one 416-wide ACT copy per pair) -> bispectrum
        split across DVE (pair terms) + GPSIMD (n1=0 terms) -> bias+relu
        -> out DMA in 4-row slices on alternating queues.
"""
import os
import sys
import types
from itertools import product

import numpy as np

sys.path.insert(0, "/opt/trn_rl_repo")
sys.path.insert(0, "/root/.axon_site")

import concourse.bass as bass
import concourse.bacc as bacc
import concourse.tile as tile
from concourse import mybir
from concourse import bass_utils

# ---------------- problem constants ----------------
KS, MD, STREAMS, C_IN = 7, 6, 16, 32
H = W = 128
HO = WO = 122
NC_RE = (MD + 1) * STREAMS       # 112
NCONV = 208                      # re[0..6] (112) + im[1..6] (96); im0 == 0
IM_BASE = NC_RE - 16             # im[n] at IM_BASE + n*16 for n >= 1
NB = 8                           # batch == cores
NPAIR = HO // 2                  # 61 psum row-pairs

# ---------------- tuning knobs ----------------
CONV_DT = os.environ.get("CONV_DT", "f16")       # f32r | f16 | bf16
R = int(os.environ.get("BISP_R", "16"))          # rows per bispectrum group
WARM = int(os.environ.get("WARM", "10"))         # warmup matmul count
TAILW = int(os.environ.get("TAILW", "0"))      # tail-warming matmul count

F16 = mybir.dt.float16
_DT_MAP = {"f32r": mybir.dt.float32r, "f16": mybir.dt.float16,
           "bf16": mybir.dt.bfloat16}
_NP_MAP = {"f32r": np.float32, "f16": np.float16, "bf16": None}


def _np_conv_dtype():
    if CONV_DT == "bf16":
        import ml_dtypes
        return ml_dtypes.bfloat16
    return _NP_MAP[CONV_DT]


# ---------------- host-side filter construction ----------------
def _tri(v):
    return np.where(np.abs(v) <= 1, np.where(v < 0, v + 1, 1 - v), 0)


def _make_atoms(kernel_size, max_degree):
    radius = (kernel_size - 1) // 2
    g = np.arange(-radius, radius + 1)
    xg, yg = np.meshgrid(g, g)
    r = np.sqrt(xg ** 2 + yg ** 2)
    theta = np.arctan2(yg, xg)
    n_rp = kernel_size // 2 + 1
    atoms = np.zeros((kernel_size, kernel_size, max_degree + 1, n_rp),
                     dtype=np.complex64)
    for i, n in product(range(n_rp), range(max_degree + 1)):
        atoms[:, :, n, i] = _tri(r - i) * np.exp(theta * n * -1j)
    atoms[kernel_size // 2, kernel_size // 2, 1:, :] = 0
    norm = np.sqrt(np.sum(np.conj(atoms) * atoms, axis=(0, 1)))
    norm[norm == 0] = 1
    return (atoms / norm).astype(np.complex64)


_ATOMS = _make_atoms(KS, MD)


def _host_filters(w):
    """w (1,1,32,16,7,4) -> (fdA [128, 7*208], fdB [96, 7*208]) float32.
    Block j (208 cols) holds kh = 6-j so that [f(kh) | f(kh-1)] is a
    contiguous 416-col slice. Channel order within a block: col n*16+s =
    re(n,s); 112+(n-1)*16+s = im(n,s), n>=1."""
    wc = w[0, 0]
    f_re = np.einsum("hwnr,csnr->hwcsn", _ATOMS.real, wc)
    f_im = np.einsum("hwnr,csnr->hwcsn", _ATOMS.imag, wc)
    filt = np.zeros((KS, KS, C_IN, NCONV), np.float32)
    filt[:, :, :, 0:NC_RE] = np.transpose(f_re, (0, 1, 2, 4, 3)).reshape(
        KS, KS, C_IN, NC_RE)
    filt[:, :, :, NC_RE:NCONV] = np.transpose(
        f_im[:, :, :, :, 1:], (0, 1, 2, 4, 3)).reshape(KS, KS, C_IN, 96)
    # 8 blocks: block 7 is zeros so a full-width (416-col) matmul with
    # start=True can reset a whole PSUM pair bank while only the left
    # half receives real filter data.
    fdA = np.zeros((128, KS + 1, NCONV), np.float32)
    fdB = np.zeros((96, KS + 1, NCONV), np.float32)
    for kh in range(KS):
        fdA[:, 6 - kh, :] = filt[kh, 0:4].reshape(128, NCONV)
        fdB[:, 6 - kh, :] = filt[kh, 4:7].reshape(96, NCONV)
    return fdA.reshape(128, (KS + 1) * NCONV), fdB.reshape(
        96, (KS + 1) * NCONV)


# ---------------- bass program ----------------
def _ap(src_ap, off, dims):
    """New AP into the same tensor: explicit [step, count] dims (elements)."""
    return bass.AP(tensor=src_ap.tensor, offset=src_ap.offset + off, ap=dims)


_PROGRAM = None


def _build_program():
    cdt = _DT_MAP[CONV_DT]
    nc = bacc.Bacc("TRN2", target_bir_lowering=False, debug=False,
                   num_devices=NB)
    sA_d = nc.dram_tensor("im2rowA", [128, H * WO], cdt,
                          kind="ExternalInput").ap()
    sB_d = nc.dram_tensor("im2rowB", [96, H * WO], cdt,
                          kind="ExternalInput").ap()
    fA_d = nc.dram_tensor("filtA", [128, (KS + 1) * NCONV], cdt,
                          kind="ExternalInput").ap()
    fB_d = nc.dram_tensor("filtB", [96, (KS + 1) * NCONV], cdt,
                          kind="ExternalInput").ap()
    bias_d = nc.dram_tensor("biasrep", [128, R * 256], F16,
                            kind="ExternalInput").ap()
    # w-major output layout: contiguous (h, c) runs per w-partition
    out_d = nc.dram_tensor("out", [WO, HO, 256], F16,
                           kind="ExternalOutput").ap()
    MUL = mybir.AluOpType.mult
    ADD = mybir.AluOpType.add
    SUB = mybir.AluOpType.subtract

    with tile.TileContext(nc) as tc:
        with tc.tile_pool(name="const", bufs=1) as constp, \
             tc.tile_pool(name="stk", bufs=5) as stkp, \
             tc.tile_pool(name="fm", bufs=4) as fmp, \
             tc.tile_pool(name="tmp", bufs=2) as tmpp, \
             tc.tile_pool(name="yp", bufs=3) as yp, \
             tc.tile_pool(name="ps", bufs=8, space="PSUM") as psp:

            # ---- warmup tile (no DMA dependency): PE clock ramp ----
            wz = constp.tile([128, NCONV], cdt, name="wz")
            nc.gpsimd.memset(wz[:], 0.0)
            wps = psp.tile([128, 416], mybir.dt.float32, tag="ps",
                           name="warmps")
            for _ in range(WARM):
                nc.tensor.matmul(wps[0:WO, 0:NCONV], wz[:, 0:WO], wz[:],
                                 start=True, stop=True)

            # ---- constants ----
            fAllT = constp.tile([128, (KS + 1) * NCONV], cdt, name="fAll")
            fBllT = constp.tile([96, (KS + 1) * NCONV], cdt, name="fBll")
            nc.scalar.dma_start(fAllT[:], fA_d[:, :])
            nc.scalar.dma_start(fBllT[:], fB_d[:, :])
            biasT = constp.tile([128, R * 256], F16)
            nc.scalar.dma_start(biasT[:], bias_d[:])

            psum_by_m = {}
            group = {}   # current bispectrum group state
            # groups (start_row, nrows): taper shrinks the tail chain
            GDEF = [(0, 16), (16, 16), (32, 16), (48, 16), (64, 16),
                    (80, 16), (96, 8), (104, 8), (112, 6), (118, 4)]
            P2G = {}     # pair index -> (group idx, pair-within-group)
            for gi, (s, n) in enumerate(GDEF):
                for j in range(n // 2):
                    P2G[s // 2 + j] = (gi, j)

            def sview(t, off, dims, nparts=WO):
                a = t[:]
                return bass.AP(tensor=a.tensor, offset=a.offset + off,
                               ap=[[a.ap[0][0], nparts]] + dims)

            dma_rr = [0]

            def bisp(fmT, h0, Rg):
                """Bispectrum for rows h0..h0+Rg-1; fm read directly via
                run-decomposed (stride-0 broadcast) APs - no gathers.
                DVE computes the (n1>=1) pair block (y cols 64:256);
                GPSIMD computes the n1=0 block (y cols 0:64)."""
                def fmr(comp, n0, cnt):     # contiguous n-run view
                    return sview(fmT, comp + n0 * 16,
                                 [[NCONV, Rg], [1, cnt * 16]])

                def fmb(comp, n, reps):     # broadcast single-n view
                    return sview(fmT, comp + n * 16,
                                 [[NCONV, Rg], [0, reps], [1, 16]])

                t1 = tmpp.tile([128, R * 96], F16, tag="t1")
                t2 = tmpp.tile([128, R * 96], F16, tag="t2")
                t3 = tmpp.tile([128, R * 96], F16, tag="t3")
                t4 = tmpp.tile([128, R * 96], F16, tag="t4")
                t5 = tmpp.tile([128, R * 96], F16, tag="t5")
                t6 = tmpp.tile([128, R * 96], F16, tag="t6")

                def tv(t, p0, L):
                    return sview(t, p0 * 16, [[96, Rg], [1, L]])
                full = lambda t: t[0:WO, 0:Rg * 96]
                RE, IM = 0, IM_BASE
                # runs: (pair0, cnt, nA, nB0, nC0)
                RUNS = [(0, 3, 1, 1, 2), (3, 2, 2, 2, 4), (5, 1, 3, 3, 6)]
                y = yp.tile([128, R * 256], F16, tag="y")
                # stage 1: re1 = ArBr - AiBi -> t1 ; im1 = ArBi + AiBr -> t3
                for (p0, cnt, na, nb, ncn) in RUNS:
                    L = cnt * 16
                    nc.vector.tensor_tensor(tv(t1, p0, L), fmb(RE, na, cnt),
                                            fmr(RE, nb, cnt), MUL)
                    nc.vector.tensor_tensor(tv(t2, p0, L), fmb(IM, na, cnt),
                                            fmr(IM, nb, cnt), MUL)
                    nc.vector.tensor_tensor(tv(t3, p0, L), fmb(RE, na, cnt),
                                            fmr(IM, nb, cnt), MUL)
                    nc.vector.tensor_tensor(tv(t4, p0, L), fmb(IM, na, cnt),
                                            fmr(RE, nb, cnt), MUL)
                nc.vector.tensor_tensor(full(t1), full(t1), full(t2), SUB)
                nc.vector.tensor_tensor(full(t3), full(t3), full(t4), ADD)
                # stage 2
                yv_re = sview(y, 64, [[256, Rg], [32, 6], [1, 16]])
                yv_im = sview(y, 80, [[256, Rg], [32, 6], [1, 16]])
                for (p0, cnt, na, nb, ncn) in RUNS:
                    L = cnt * 16
                    nc.vector.tensor_tensor(tv(t2, p0, L), tv(t1, p0, L),
                                            fmr(RE, ncn, cnt), MUL)
                    nc.vector.tensor_tensor(tv(t4, p0, L), tv(t3, p0, L),
                                            fmr(IM, ncn, cnt), MUL)
                nc.vector.tensor_tensor(yv_re, full(t2), full(t4), ADD)
                for (p0, cnt, na, nb, ncn) in RUNS:
                    L = cnt * 16
                    nc.vector.tensor_tensor(tv(t5, p0, L), tv(t3, p0, L),
                                            fmr(RE, ncn, cnt), MUL)
                    nc.vector.tensor_tensor(tv(t6, p0, L), tv(t1, p0, L),
                                            fmr(IM, ncn, cnt), MUL)
                nc.vector.tensor_tensor(yv_im, full(t5), full(t6), SUB)
                # (0,n): y[16:64] = re0 * (re(n)^2 + im(n)^2), n=1..3
                s1 = tmpp.tile([128, R * 48], F16, tag="s1")
                s2 = tmpp.tile([128, R * 48], F16, tag="s2")
                nc.scalar.square(s1[0:WO, 0:Rg * 48], fmr(RE, 1, 3))
                nc.scalar.square(s2[0:WO, 0:Rg * 48], fmr(IM, 1, 3))
                nc.vector.tensor_tensor(s1[0:WO, 0:Rg * 48],
                                        s1[0:WO, 0:Rg * 48],
                                        s2[0:WO, 0:Rg * 48], ADD)
                nc.vector.tensor_tensor(
                    sview(y, 16, [[256, Rg], [1, 48]]),
                    s1[0:WO, 0:Rg * 48], fmb(RE, 0, 3), MUL)
                # (0,0): y[0:16] = re0^3
                s3 = tmpp.tile([128, R * 16], F16, tag="s3")
                nc.scalar.square(s3[0:WO, 0:Rg * 16], fmr(RE, 0, 1))
                nc.vector.tensor_tensor(
                    sview(y, 0, [[256, Rg], [1, 16]]),
                    s3[0:WO, 0:Rg * 16], fmr(RE, 0, 1), MUL)
                # bias + relu + output; bulk groups use 4-row slices on
                # scalar/gpsimd; tail groups use 2-row slices spread over
                # all three queues so the final transfers drain in parallel
                late = h0 >= 96
                step = 2 if late else 4
                for j0 in range(0, Rg, step):
                    j1 = min(j0 + step, Rg)
                    L = (j1 - j0) * 256
                    nc.vector.tensor_tensor(y[0:WO, j0 * 256:j1 * 256],
                                            y[0:WO, j0 * 256:j1 * 256],
                                            biasT[0:WO, 0:L], ADD)
                    nc.vector.tensor_scalar_max(y[0:WO, j0 * 256:j1 * 256],
                                                y[0:WO, j0 * 256:j1 * 256],
                                                0.0)
                    if late:
                        eng = (nc.sync, nc.scalar, nc.gpsimd)[dma_rr[0] % 3]
                    else:
                        eng = (nc.scalar, nc.gpsimd)[dma_rr[0] % 2]
                    dma_rr[0] += 1
                    eng.dma_start(
                        _ap(out_d, (h0 + j0) * 256,
                            [[HO * 256, WO], [1, (j1 - j0) * 256]]),
                        y[0:WO, j0 * 256:j1 * 256])

            # ---- main loop over input rows ----
            RB = 32     # input rows per stack-DMA block (kw-split DMAs)
            NBLK = H // RB
            sA_t = {}
            sB_t = {}

            def emit_stack_block(r0, nr, tag):
                sAb = stkp.tile([128, nr * WO], cdt, tag=f"sA{tag}",
                                name=f"sA{r0}")
                sBb = stkp.tile([96, nr * WO], cdt, tag=f"sB{tag}",
                                name=f"sB{r0}")
                nc.sync.dma_start(
                    sAb[:],
                    _ap(sA_d, r0 * WO, [[H * WO, 128], [1, nr * WO]]))
                nc.sync.dma_start(
                    sBb[:],
                    _ap(sB_d, r0 * WO, [[H * WO, 96], [1, nr * WO]]))
                for j in range(nr):
                    sA_t[r0 + j] = (sAb, j)
                    sB_t[r0 + j] = (sBb, j)

            for (r0, nr) in ((0, 4), (4, 4), (8, 8), (16, 8), (24, 8)):
                emit_stack_block(r0, nr, "f")     # fast start, fine blocks
            for blk in range(1, NBLK):            # rows 32+, coarse
                emit_stack_block(blk * RB, RB, "c")

            def conv_row(r):
                sA = sA_t[r][0][:, sA_t[r][1] * WO:(sA_t[r][1] + 1) * WO]
                sB = sB_t[r][0][:, sB_t[r][1] * WO:(sB_t[r][1] + 1) * WO]
                ms = {}
                for kh in range(KS):
                    h = r - kh
                    if 0 <= h < HO:
                        ms.setdefault(h >> 1, []).append(kh)
                for part, stk, fall in ((0, sA, fAllT), (1, sB, fBllT)):
                    for m in sorted(ms):
                        if m not in psum_by_m:
                            psum_by_m[m] = psp.tile(
                                [128, 416], mybir.dt.float32, tag="ps",
                                name=f"ps{m}")
                        pt = psum_by_m[m]
                        khs = ms[m]

                        def emit(reg, kh, start, stop):
                            if reg == 'D':
                                o = pt[0:WO, 0:416]
                                mv = fall[:, (6 - kh) * NCONV:
                                          (8 - kh) * NCONV]
                            elif reg == 'L':
                                o = pt[0:WO, 0:NCONV]
                                mv = fall[:, (6 - kh) * NCONV:
                                          (7 - kh) * NCONV]
                            else:
                                o = pt[0:WO, NCONV:416]
                                mv = fall[:, (6 - kh) * NCONV:
                                          (7 - kh) * NCONV]
                            nc.tensor.matmul(o, stk, mv, start=start,
                                             stop=stop,
                                             skip_group_check=True)

                        if len(khs) == 2:
                            kh1, kh0 = khs
                            emit('D', kh0, False, False)
                        elif khs[0] == 0:
                            # r == 2m: pair's first contribution. start=True
                            # on the A single resets the WHOLE psum bank
                            # (hardware wipes the full bank, zeroing the R
                            # half for free); everything after accumulates.
                            emit('L', 0, part == 0, False)
                        else:
                            # r == 2m+7: kh == 6, last touch (R half)
                            emit('R', 6, False, part == 1)

            def pair_complete(m):
                gi, j = P2G[m]
                if j == 0:
                    group["fm"] = fmp.tile([128, R * NCONV], F16,
                                           tag="fm", name=f"fm{m}")
                    group["h0"] = m * 2
                    group["Rg"] = GDEF[gi][1]
                ps = psum_by_m.pop(m)
                with tc.high_priority():
                    nc.scalar.copy(
                        group["fm"][0:WO, 2 * j * NCONV:(2 * j + 2) * NCONV],
                        ps[0:WO, 0:416])
                if j == GDEF[gi][1] // 2 - 1:
                    bisp(group["fm"], group["h0"], group["Rg"])

            for r in range(H):
                conv_row(r)
                if r >= KS and (r & 1) == 1:
                    pair_complete((r - KS) // 2)
            # tail warming: keep the PE active so HAM holds full clock
            # while the last bispectrum groups drain on DVE/ACT
            if TAILW:
                tps = psp.tile([128, 416], mybir.dt.float32, tag="ps",
                               name="tailps")
                for _ in range(TAILW):
                    nc.tensor.matmul(tps[0:WO, 0:32], wz[:, 0:WO],
                                     wz[:, 0:32], start=True, stop=True)
    nc.compile()
    return nc


def _get_program():
    global _PROGRAM
    if _PROGRAM is None:
        _PROGRAM = _build_program()
    return _PROGRAM


def _install_trace_shim():
    """antenv.axon_hooks is absent in this image; recreate via ctypes."""
    if "antenv.axon_hooks" in sys.modules:
        return
    try:
        from trn_agent_boot.trn_boot import _ntff_profile_via_ctypes
        hook = _ntff_profile_via_ctypes("/opt/axon/libaxon_pjrt.so")
    except Exception:
        hook = None
    m = types.ModuleType("antenv.axon_hooks")
    m.get_axon_ntff_profile_hook = lambda: hook
    m.set_axon_ntff_profile_hook = lambda h: None
    sys.modules["antenv.axon_hooks"] = m
    bass_utils.upload_artifacts = lambda tmpdir: tmpdir


def kernel(x, w, bias, _trace=False, _tmpdir=None):
    """Full inputs -> full output (8,122,122,256) float32."""
    x = np.asarray(x, dtype=np.float32)
    w = np.asarray(w, dtype=np.float32)
    bias = np.asarray(bias, dtype=np.float32)
    np_cdt = _np_conv_dtype()

    fdA, fdB = _host_filters(w)
    fdA = fdA.astype(np_cdt)
    fdB = fdB.astype(np_cdt)
    biasrep = np.broadcast_to(
        np.tile(bias.astype(np.float16), R)[None, :],
        (128, R * 256)).copy()
    in_maps = []
    xc = x.astype(np_cdt)           # (B, H, W, C) fp16
    for b in range(NB):
        xb = xc[b]
        st = np.lib.stride_tricks.as_strided(
            xb, shape=(KS, C_IN, H, WO),
            strides=(xb.strides[1], xb.strides[2],
                     xb.strides[0], xb.strides[1]))
        sa = np.ascontiguousarray(st[0:4]).reshape(128, H * WO)
        sb = np.ascontiguousarray(st[4:7]).reshape(96, H * WO)
        in_maps.append({"im2rowA": sa, "im2rowB": sb, "filtA": fdA,
                        "filtB": fdB, "biasrep": biasrep})

    nc = _get_program()
    kwargs = {}
    if _trace:
        _install_trace_shim()
        kwargs = dict(trace=True, tmpdir=_tmpdir)
    res = bass_utils.run_bass_kernel_spmd(nc, in_maps,
                                          core_ids=list(range(NB)), **kwargs)
    out = np.stack([res.results[b]["out"].transpose(1, 0, 2)
                    for b in range(NB)], axis=0).astype(np.float32)
    if _trace:
        return out, res
    return out


if __name__ == "__main__":
    d = np.load("/tmp/ref_io.npz")
    out = kernel(d["x"], d["w"], d["bias"])
    exp = d["expected"]
    rel = np.linalg.norm(out - exp) / np.linalg.norm(exp)
    print("rel_l2 =", rel)


# revision 23
# speedup vs baseline: 1.0123x; 1.0123x over previous
"""Trainium2 Bass kernel for nn_BCHConv2D (complex harmonic conv + bispectrum).

Strategy (8 NeuronCores, data-parallel over batch B=8):
  host: build complex-harmonic filters from w+atoms -> fdA [128, 7*208],
        fdB [96, 7*208] (kh-reversed contiguous blocks), transpose each
        batch image to (H, C, W), replicate bias.
  core: 7x7x32 -> 208ch conv; PSUM banks hold PAIRS of output rows
        [122, 2*208]; per input row the two kh-adjacent filter blocks are
        streamed as one 416-wide moving operand (halves matmul count).
        PSUM -> fp16 SBUF (one 416-wide ACT copy per pair) -> bispectrum
        split across DVE (pair terms) + GPSIMD (n1=0 terms) -> bias+relu
        -> out DMA in 4-row slices on alternating queues.
"""
import os
import sys
import types
from itertools import product

import numpy as np

sys.path.insert(0, "/opt/trn_rl_repo")
sys.path.insert(0, "/root/.axon_site")

import concourse.bass as bass
import concourse.bacc as bacc
import concourse.tile as tile
from concourse import mybir
from concourse import bass_utils

# ---------------- problem constants ----------------
KS, MD, STREAMS, C_IN = 7, 6, 16, 32
H = W = 128
HO = WO = 122
NC_RE = (MD + 1) * STREAMS       # 112
NCONV = 208                      # re[0..6] (112) + im[1..6] (96); im0 == 0
IM_BASE = NC_RE - 16             # im[n] at IM_BASE + n*16 for n >= 1
NB = 8                           # batch == cores
NPAIR = HO // 2                  # 61 psum row-pairs

# ---------------- tuning knobs ----------------
CONV_DT = os.environ.get("CONV_DT", "f16")       # f32r | f16 | bf16
R = int(os.environ.get("BISP_R", "16"))          # rows per bispectrum group
WARM = int(os.environ.get("WARM", "10"))         # warmup matmul count
TAILW = int(os.environ.get("TAILW", "0"))      # tail-warming matmul count

F16 = mybir.dt.float16
_DT_MAP = {"f32r": mybir.dt.float32r, "f16": mybir.dt.float16,
           "bf16": mybir.dt.bfloat16}
_NP_MAP = {"f32r": np.float32, "f16": np.float16, "bf16": None}


def _np_conv_dtype():
    if CONV_DT == "bf16":
        import ml_dtypes
        return ml_dtypes.bfloat16
    return _NP_MAP[CONV_DT]


# ---------------- host-side filter construction ----------------
def _tri(v):
    return np.where(np.abs(v) <= 1, np.where(v < 0, v + 1, 1 - v), 0)


def _make_atoms(kernel_size, max_degree):
    radius = (kernel_size - 1) // 2
    g = np.arange(-radius, radius + 1)
    xg, yg = np.meshgrid(g, g)
    r = np.sqrt(xg ** 2 + yg ** 2)
    theta = np.arctan2(yg, xg)
    n_rp = kernel_size // 2 + 1
    atoms = np.zeros((kernel_size, kernel_size, max_degree + 1, n_rp),
                     dtype=np.complex64)
    for i, n in product(range(n_rp), range(max_degree + 1)):
        atoms[:, :, n, i] = _tri(r - i) * np.exp(theta * n * -1j)
    atoms[kernel_size // 2, kernel_size // 2, 1:, :] = 0
    norm = np.sqrt(np.sum(np.conj(atoms) * atoms, axis=(0, 1)))
    norm[norm == 0] = 1
    return (atoms / norm).astype(np.complex64)


_ATOMS = _make_atoms(KS, MD)


def _host_filters(w):
    """w (1,1,32,16,7,4) -> (fdA [128, 7*208], fdB [96, 7*208]) float32.
    Block j (208 cols) holds kh = 6-j so that [f(kh) | f(kh-1)] is a
    contiguous 416-col slice. Channel order within a block: col n*16+s =
    re(n,s); 112+(n-1)*16+s = im(n,s), n>=1."""
    wc = w[0, 0]
    f_re = np.einsum("hwnr,csnr->hwcsn", _ATOMS.real, wc)
    f_im = np.einsum("hwnr,csnr->hwcsn", _ATOMS.imag, wc)
    filt = np.zeros((KS, KS, C_IN, NCONV), np.float32)
    filt[:, :, :, 0:NC_RE] = np.transpose(f_re, (0, 1, 2, 4, 3)).reshape(
        KS, KS, C_IN, NC_RE)
    filt[:, :, :, NC_RE:NCONV] = np.transpose(
        f_im[:, :, :, :, 1:], (0, 1, 2, 4, 3)).reshape(KS, KS, C_IN, 96)
    # 8 blocks: block 7 is zeros so a full-width (416-col) matmul with
    # start=True can reset a whole PSUM pair bank while only the left
    # half receives real filter data.
    fdA = np.zeros((128, KS + 1, NCONV), np.float32)
    fdB = np.zeros((96, KS + 1, NCONV), np.float32)
    for kh in range(KS):
        fdA[:, 6 - kh, :] = filt[kh, 0:4].reshape(128, NCONV)
        fdB[:, 6 - kh, :] = filt[kh, 4:7].reshape(96, NCONV)
    return fdA.reshape(128, (KS + 1) * NCONV), fdB.reshape(
        96, (KS + 1) * NCONV)


# ---------------- bass program ----------------
def _ap(src_ap, off, dims):
    """New AP into the same tensor: explicit [step, count] dims (elements)."""
    return bass.AP(tensor=src_ap.tensor, offset=src_ap.offset + off, ap=dims)


_PROGRAM = None


def _build_program():
    cdt = _DT_MAP[CONV_DT]
    nc = bacc.Bacc("TRN2", target_bir_lowering=False, debug=False,
                   num_devices=NB)
    sA_d = nc.dram_tensor("im2rowA", [128, H * WO], cdt,
                          kind="ExternalInput").ap()
    sB_d = nc.dram_tensor("im2rowB", [96, H * WO], cdt,
                          kind="ExternalInput").ap()
    fA_d = nc.dram_tensor("filtA", [128, (KS + 1) * NCONV], cdt,
                          kind="ExternalInput").ap()
    fB_d = nc.dram_tensor("filtB", [96, (KS + 1) * NCONV], cdt,
                          kind="ExternalInput").ap()
    bias_d = nc.dram_tensor("biasrep", [128, R * 256], F16,
                            kind="ExternalInput").ap()
    # w-major output layout: contiguous (h, c) runs per w-partition
    out_d = nc.dram_tensor("out", [WO, HO, 256], F16,
                           kind="ExternalOutput").ap()
    MUL = mybir.AluOpType.mult
    ADD = mybir.AluOpType.add
    SUB = mybir.AluOpType.subtract

    with tile.TileContext(nc) as tc:
        with tc.tile_pool(name="const", bufs=1) as constp, \
             tc.tile_pool(name="stk", bufs=5) as stkp, \
             tc.tile_pool(name="fm", bufs=4) as fmp, \
             tc.tile_pool(name="tmp", bufs=2) as tmpp, \
             tc.tile_pool(name="yp", bufs=3) as yp, \
             tc.tile_pool(name="ps", bufs=8, space="PSUM") as psp:

            # ---- warmup tile (no DMA dependency): PE clock ramp ----
            wz = constp.tile([128, NCONV], cdt, name="wz")
            nc.gpsimd.memset(wz[:], 0.0)
            wps = psp.tile([128, 416], mybir.dt.float32, tag="ps",
                           name="warmps")
            for _ in range(WARM):
                nc.tensor.matmul(wps[0:WO, 0:NCONV], wz[:, 0:WO], wz[:],
                                 start=True, stop=True)

            # ---- constants ----
            fAllT = constp.tile([128, (KS + 1) * NCONV], cdt, name="fAll")
            fBllT = constp.tile([96, (KS + 1) * NCONV], cdt, name="fBll")
            nc.scalar.dma_start(fAllT[:], fA_d[:, :])
            nc.scalar.dma_start(fBllT[:], fB_d[:, :])
            biasT = constp.tile([128, R * 256], F16)
            nc.scalar.dma_start(biasT[:], bias_d[:])

            psum_by_m = {}
            group = {}   # current bispectrum group state
            # groups (start_row, nrows): taper shrinks the tail chain
            GDEF = [(0, 16), (16, 16), (32, 16), (48, 16), (64, 16),
                    (80, 16), (96, 8), (104, 8), (112, 4), (116, 4),
                    (120, 2)]
            P2G = {}     # pair index -> (group idx, pair-within-group)
            for gi, (s, n) in enumerate(GDEF):
                for j in range(n // 2):
                    P2G[s // 2 + j] = (gi, j)

            def sview(t, off, dims, nparts=WO):
                a = t[:]
                return bass.AP(tensor=a.tensor, offset=a.offset + off,
                               ap=[[a.ap[0][0], nparts]] + dims)

            dma_rr = [0]

            def bisp(fmT, h0, Rg):
                """Bispectrum for rows h0..h0+Rg-1; fm read directly via
                run-decomposed (stride-0 broadcast) APs - no gathers.
                DVE computes the (n1>=1) pair block (y cols 64:256);
                GPSIMD computes the n1=0 block (y cols 0:64)."""
                def fmr(comp, n0, cnt):     # contiguous n-run view
                    return sview(fmT, comp + n0 * 16,
                                 [[NCONV, Rg], [1, cnt * 16]])

                def fmb(comp, n, reps):     # broadcast single-n view
                    return sview(fmT, comp + n * 16,
                                 [[NCONV, Rg], [0, reps], [1, 16]])

                t1 = tmpp.tile([128, R * 96], F16, tag="t1")
                t2 = tmpp.tile([128, R * 96], F16, tag="t2")
                t3 = tmpp.tile([128, R * 96], F16, tag="t3")
                t4 = tmpp.tile([128, R * 96], F16, tag="t4")
                t5 = tmpp.tile([128, R * 96], F16, tag="t5")
                t6 = tmpp.tile([128, R * 96], F16, tag="t6")

                def tv(t, p0, L):
                    return sview(t, p0 * 16, [[96, Rg], [1, L]])
                full = lambda t: t[0:WO, 0:Rg * 96]
                RE, IM = 0, IM_BASE
                # runs: (pair0, cnt, nA, nB0, nC0)
                RUNS = [(0, 3, 1, 1, 2), (3, 2, 2, 2, 4), (5, 1, 3, 3, 6)]
                y = yp.tile([128, R * 256], F16, tag="y")
                # stage 1: re1 = ArBr - AiBi -> t1 ; im1 = ArBi + AiBr -> t3
                for (p0, cnt, na, nb, ncn) in RUNS:
                    L = cnt * 16
                    nc.vector.tensor_tensor(tv(t1, p0, L), fmb(RE, na, cnt),
                                            fmr(RE, nb, cnt), MUL)
                    nc.vector.tensor_tensor(tv(t2, p0, L), fmb(IM, na, cnt),
                                            fmr(IM, nb, cnt), MUL)
                    nc.vector.tensor_tensor(tv(t3, p0, L), fmb(RE, na, cnt),
                                            fmr(IM, nb, cnt), MUL)
                    nc.vector.tensor_tensor(tv(t4, p0, L), fmb(IM, na, cnt),
                                            fmr(RE, nb, cnt), MUL)
                nc.vector.tensor_tensor(full(t1), full(t1), full(t2), SUB)
                nc.vector.tensor_tensor(full(t3), full(t3), full(t4), ADD)
                # stage 2
                yv_re = sview(y, 64, [[256, Rg], [32, 6], [1, 16]])
                yv_im = sview(y, 80, [[256, Rg], [32, 6], [1, 16]])
                for (p0, cnt, na, nb, ncn) in RUNS:
                    L = cnt * 16
                    nc.vector.tensor_tensor(tv(t2, p0, L), tv(t1, p0, L),
                                            fmr(RE, ncn, cnt), MUL)
                    nc.vector.tensor_tensor(tv(t4, p0, L), tv(t3, p0, L),
                                            fmr(IM, ncn, cnt), MUL)
                nc.vector.tensor_tensor(yv_re, full(t2), full(t4), ADD)
                for (p0, cnt, na, nb, ncn) in RUNS:
                    L = cnt * 16
                    nc.vector.tensor_tensor(tv(t5, p0, L), tv(t3, p0, L),
                                            fmr(RE, ncn, cnt), MUL)
                    nc.vector.tensor_tensor(tv(t6, p0, L), tv(t1, p0, L),
                                            fmr(IM, ncn, cnt), MUL)
                nc.vector.tensor_tensor(yv_im, full(t5), full(t6), SUB)
                # (0,n): y[16:64] = re0 * (re(n)^2 + im(n)^2), n=1..3
                s1 = tmpp.tile([128, R * 48], F16, tag="s1")
                s2 = tmpp.tile([128, R * 48], F16, tag="s2")
                nc.scalar.square(s1[0:WO, 0:Rg * 48], fmr(RE, 1, 3))
                nc.scalar.square(s2[0:WO, 0:Rg * 48], fmr(IM, 1, 3))
                nc.vector.tensor_tensor(s1[0:WO, 0:Rg * 48],
                                        s1[0:WO, 0:Rg * 48],
                                        s2[0:WO, 0:Rg * 48], ADD)
                nc.vector.tensor_tensor(
                    sview(y, 16, [[256, Rg], [1, 48]]),
                    s1[0:WO, 0:Rg * 48], fmb(RE, 0, 3), MUL)
                # (0,0): y[0:16] = re0^3
                s3 = tmpp.tile([128, R * 16], F16, tag="s3")
                nc.scalar.square(s3[0:WO, 0:Rg * 16], fmr(RE, 0, 1))
                nc.vector.tensor_tensor(
                    sview(y, 0, [[256, Rg], [1, 16]]),
                    s3[0:WO, 0:Rg * 16], fmr(RE, 0, 1), MUL)
                # bias + relu + output; bulk groups use 4-row slices on
                # scalar/gpsimd; tail groups use 2-row slices spread over
                # all three queues so the final transfers drain in parallel
                late = h0 >= 96
                step = 2 if late else 4
                for j0 in range(0, Rg, step):
                    j1 = min(j0 + step, Rg)
                    L = (j1 - j0) * 256
                    nc.vector.tensor_tensor(y[0:WO, j0 * 256:j1 * 256],
                                            y[0:WO, j0 * 256:j1 * 256],
                                            biasT[0:WO, 0:L], ADD)
                    nc.vector.tensor_scalar_max(y[0:WO, j0 * 256:j1 * 256],
                                                y[0:WO, j0 * 256:j1 * 256],
                                                0.0)
                    if late:
                        eng = (nc.sync, nc.scalar, nc.gpsimd)[dma_rr[0] % 3]
                    else:
                        eng = (nc.scalar, nc.gpsimd)[dma_rr[0] % 2]
                    dma_rr[0] += 1
                    eng.dma_start(
                        _ap(out_d, (h0 + j0) * 256,
                            [[HO * 256, WO], [1, (j1 - j0) * 256]]),
                        y[0:WO, j0 * 256:j1 * 256])

            # ---- main loop over input rows ----
            RB = 32     # input rows per stack-DMA block (kw-split DMAs)
            NBLK = H // RB
            sA_t = {}
            sB_t = {}

            def emit_stack_block(r0, nr, tag):
                sAb = stkp.tile([128, nr * WO], cdt, tag=f"sA{tag}",
                                name=f"sA{r0}")
                sBb = stkp.tile([96, nr * WO], cdt, tag=f"sB{tag}",
                                name=f"sB{r0}")
                nc.sync.dma_start(
                    sAb[:],
                    _ap(sA_d, r0 * WO, [[H * WO, 128], [1, nr * WO]]))
                nc.sync.dma_start(
                    sBb[:],
                    _ap(sB_d, r0 * WO, [[H * WO, 96], [1, nr * WO]]))
                for j in range(nr):
                    sA_t[r0 + j] = (sAb, j)
                    sB_t[r0 + j] = (sBb, j)

            for (r0, nr) in ((0, 4), (4, 4), (8, 8), (16, 8), (24, 8)):
                emit_stack_block(r0, nr, "f")     # fast start, fine blocks
            for blk in range(1, NBLK):            # rows 32+, coarse
                emit_stack_block(blk * RB, RB, "c")

            def conv_row(r):
                sA = sA_t[r][0][:, sA_t[r][1] * WO:(sA_t[r][1] + 1) * WO]
                sB = sB_t[r][0][:, sB_t[r][1] * WO:(sB_t[r][1] + 1) * WO]
                ms = {}
                for kh in range(KS):
                    h = r - kh
                    if 0 <= h < HO:
                        ms.setdefault(h >> 1, []).append(kh)
                for part, stk, fall in ((0, sA, fAllT), (1, sB, fBllT)):
                    for m in sorted(ms):
                        if m not in psum_by_m:
                            psum_by_m[m] = psp.tile(
                                [128, 416], mybir.dt.float32, tag="ps",
                                name=f"ps{m}")
                        pt = psum_by_m[m]
                        khs = ms[m]

                        def emit(reg, kh, start, stop):
                            if reg == 'D':
                                o = pt[0:WO, 0:416]
                                mv = fall[:, (6 - kh) * NCONV:
                                          (8 - kh) * NCONV]
                            elif reg == 'L':
                                o = pt[0:WO, 0:NCONV]
                                mv = fall[:, (6 - kh) * NCONV:
                                          (7 - kh) * NCONV]
                            else:
                                o = pt[0:WO, NCONV:416]
                                mv = fall[:, (6 - kh) * NCONV:
                                          (7 - kh) * NCONV]
                            nc.tensor.matmul(o, stk, mv, start=start,
                                             stop=stop,
                                             skip_group_check=True)

                        if len(khs) == 2:
                            kh1, kh0 = khs
                            emit('D', kh0, False, False)
                        elif khs[0] == 0:
                            # r == 2m: pair's first contribution. start=True
                            # on the A single resets the WHOLE psum bank
                            # (hardware wipes the full bank, zeroing the R
                            # half for free); everything after accumulates.
                            emit('L', 0, part == 0, False)
                        else:
                            # r == 2m+7: kh == 6, last touch (R half)
                            emit('R', 6, False, part == 1)

            def pair_complete(m):
                gi, j = P2G[m]
                if j == 0:
                    group["fm"] = fmp.tile([128, R * NCONV], F16,
                                           tag="fm", name=f"fm{m}")
                    group["h0"] = m * 2
                    group["Rg"] = GDEF[gi][1]
                ps = psum_by_m.pop(m)
                with tc.high_priority():
                    nc.scalar.copy(
                        group["fm"][0:WO, 2 * j * NCONV:(2 * j + 2) * NCONV],
                        ps[0:WO, 0:416])
                if j == GDEF[gi][1] // 2 - 1:
                    bisp(group["fm"], group["h0"], group["Rg"])

            for r in range(H):
                conv_row(r)
                if r >= KS and (r & 1) == 1:
                    pair_complete((r - KS) // 2)
            # tail warming: keep the PE active so HAM holds full clock
            # while the last bispectrum groups drain on DVE/ACT
            if TAILW:
                tps = psp.tile([128, 416], mybir.dt.float32, tag="ps",
                               name="tailps")
                for _ in range(TAILW):
                    nc.tensor.matmul(tps[0:WO, 0:32], wz[:, 0:WO],
                                     wz[:, 0:32], start=True, stop=True)
    nc.compile()
    return nc


def _get_program():
    global _PROGRAM
    if _PROGRAM is None:
        _PROGRAM = _build_program()
    return _PROGRAM


def _install_trace_shim():
    """antenv.axon_hooks is absent in this image; recreate via ctypes."""
    if "antenv.axon_hooks" in sys.modules:
        return
    try:
        from trn_agent_boot.trn_boot import _ntff_profile_via_ctypes
        hook = _ntff_profile_via_ctypes("/opt/axon/libaxon_pjrt.so")
    except Exception:
        hook = None
    m = types.ModuleType("antenv.axon_hooks")
    m.get_axon_ntff_profile_hook = lambda: hook
    m.set_axon_ntff_profile_hook = lambda h: None
    sys.modules["antenv.axon_hooks"] = m
    bass_utils.upload_artifacts = lambda tmpdir: tmpdir


def kernel(x, w, bias, _trace=False, _tmpdir=None):
    """Full inputs -> full output (8,122,122,256) float32."""
    x = np.asarray(x, dtype=np.float32)
    w = np.asarray(w, dtype=np.float32)
    bias = np.asarray(bias, dtype=np.float32)
    np_cdt = _np_conv_dtype()

    fdA, fdB = _host_filters(w)
    fdA = fdA.astype(np_cdt)
    fdB = fdB.astype(np_cdt)
    biasrep = np.broadcast_to(
        np.tile(bias.astype(np.float16), R)[None, :],
        (128, R * 256)).copy()
    in_maps = []
    xc = x.astype(np_cdt)           # (B, H, W, C) fp16
    for b in range(NB):
        xb = xc[b]
        st = np.lib.stride_tricks.as_strided(
            xb, shape=(KS, C_IN, H, WO),
            strides=(xb.strides[1], xb.strides[2],
                     xb.strides[0], xb.strides[1]))
        sa = np.ascontiguousarray(st[0:4]).reshape(128, H * WO)
        sb = np.ascontiguousarray(st[4:7]).reshape(96, H * WO)
        in_maps.append({"im2rowA": sa, "im2rowB": sb, "filtA": fdA,
                        "filtB": fdB, "biasrep": biasrep})

    nc = _get_program()
    kwargs = {}
    if _trace:
        _install_trace_shim()
        kwargs = dict(trace=True, tmpdir=_tmpdir)
    res = bass_utils.run_bass_kernel_spmd(nc, in_maps,
                                          core_ids=list(range(NB)), **kwargs)
    out = np.stack([res.results[b]["out"].transpose(1, 0, 2)
                    for b in range(NB)], axis=0).astype(np.float32)
    if _trace:
        return out, res
    return out


if __name__ == "__main__":
    d = np.load("/tmp/ref_io.npz")
    out = kernel(d["x"], d["w"], d["bias"])
    exp = d["expected"]
    rel = np.linalg.norm(out - exp) / np.linalg.norm(exp)
    print("rel_l2 =", rel)
